# revision 14
# baseline (speedup 1.0000x reference)
"""Trainium2 Bass kernel: causal depthwise short conv1d + SiLU.

Problem: x [B=4, T=4096, C=2048] f32, kernel [K=4, C=2048] f32.
  y[b, t, c] = silu(sum_j kernel[j, c] * x[b, t - j, c])   (zero left-pad)
  next_cache = x[:, T-K+1:, :]

Strategy:
  - Each (b, c) pair is an independent length-T sequence -> B*C = 8192 rows.
  - Shard channels across the 8 cores: core i handles channels
    [i*256, (i+1)*256) -> 1024 rows of [PAD + T] (zero pre-padded).
  - On-chip layout: partition = row, free dim = time. The 4 taps become
    per-partition tensor_scalar / scalar_tensor_tensor fused mul-adds on
    DVE/GPSIMD; SiLU runs on the scalar engine.
"""

import os
import sys

import numpy as np

_TRN_REPO = "/opt/trn_rl_repo"
if _TRN_REPO not in sys.path:
    sys.path.insert(0, _TRN_REPO)

B, T, C, K = 4, 4096, 2048, 4
PAD = K - 1
NCORES = 8
CPC = C // NCORES       # 256 channels per core
ROWS = CPC * B          # 1024 rows per core

_PROG_CACHE: dict = {}


def _build_program(use_silu: bool = True, variant: str = "split", loop: int = 1):
    """Build the per-core Bass program (identical on all 8 cores).

    loop > 1 wraps the whole body in a hardware For_i that redoes the
    (idempotent) work `loop` times — benchmarking only.
    """
    key = (use_silu, variant, loop)
    if key in _PROG_CACHE:
        return _PROG_CACHE[key]

    import concourse.tile as tile
    from concourse import bacc, mybir

    nc = bacc.Bacc("TRN2")
    xs = nc.dram_tensor("xs", [ROWS, PAD + T], mybir.dt.float32, kind="ExternalInput")
    ks = nc.dram_tensor("ks", [ROWS, K], mybir.dt.float32, kind="ExternalInput")
    ys = nc.dram_tensor("ys", [ROWS, T], mybir.dt.float32, kind="ExternalOutput")

    MULT = mybir.AluOpType.mult
    ADD = mybir.AluOpType.add
    F32 = mybir.dt.float32

    xs_ap, ks_ap, ys_ap = xs.ap(), ks.ap(), ys.ap()

    from contextlib import ExitStack

    TC = 4096 if variant == "v4" else 2048

    with tile.TileContext(nc) as tc:
        with ExitStack() as stack:
            # sim mode (use_silu=False) only checks numerics; bufs=1 keeps
            # the extra sigmoid tile inside the SBUF budget
            bx = (3 if TC == 2048 else 2) if use_silu else 1
            bt = 2 if use_silu else 1
            by = (3 if TC == 2048 else 1) if use_silu else 1
            kpool = stack.enter_context(tc.tile_pool(name="kp", bufs=2))
            xpool = stack.enter_context(tc.tile_pool(name="xp", bufs=bx))
            tpool = stack.enter_context(tc.tile_pool(name="tp", bufs=bt))
            ypool = stack.enter_context(tc.tile_pool(name="yp", bufs=by))
            if loop > 1:
                stack.enter_context(tc.For_i(0, loop, 1))
            for r in range(ROWS // 128):
                rows = slice(r * 128, (r + 1) * 128)
                k_sb = kpool.tile([128, K], F32)
                nc.sync.dma_start(k_sb[:], ks_ap[rows, :])
                for it in range(T // TC):
                    x_sb = xpool.tile([128, TC + PAD], F32)
                    nc.sync.dma_start(
                        x_sb[:], xs_ap[rows, it * TC : it * TC + TC + PAD]
                    )
                    if variant == "dmaonly":
                        nc.sync.dma_start(
                            ys_ap[rows, it * TC : (it + 1) * TC],
                            x_sb[:, PAD : PAD + TC],
                        )
                        continue
                    # z = sum_j k[:, j] * x[t - j]; x_sb col (PAD + t) = x[t]
                    def xslice(j):
                        return x_sb[:, PAD - j : PAD - j + TC]

                    def kap(j):
                        return k_sb[:, j : j + 1]

                    if variant in ("dve", "split"):
                        # chain of fused (x*k)+acc on DVE (tap2 optionally Pool)
                        t0 = tpool.tile([128, TC], F32, tag="t0")
                        nc.vector.tensor_scalar_mul(t0[:], xslice(0), kap(0))
                        t1 = tpool.tile([128, TC], F32, tag="t1")
                        nc.vector.scalar_tensor_tensor(
                            t1[:], xslice(1), kap(1), t0[:], MULT, ADD
                        )
                        t2 = tpool.tile([128, TC], F32, tag="t2")
                        eng2 = nc.gpsimd if variant == "split" else nc.vector
                        eng2.scalar_tensor_tensor(
                            t2[:], xslice(2), kap(2), t1[:], MULT, ADD
                        )
                        t3 = tpool.tile([128, TC], F32, tag="t3")
                        nc.vector.scalar_tensor_tensor(
                            t3[:], xslice(3), kap(3), t2[:], MULT, ADD
                        )
                    elif variant == "tt":
                        # 4 tensor_scalar muls + add tree, all DVE
                        ms = []
                        for j in range(K):
                            m = tpool.tile([128, TC], F32, tag=f"m{j}")
                            nc.vector.tensor_scalar_mul(m[:], xslice(j), kap(j))
                            ms.append(m)
                        s01 = tpool.tile([128, TC], F32, tag="s01")
                        nc.vector.tensor_add(s01[:], ms[0][:], ms[1][:])
                        s23 = tpool.tile([128, TC], F32, tag="s23")
                        nc.vector.tensor_add(s23[:], ms[2][:], ms[3][:])
                        t3 = tpool.tile([128, TC], F32, tag="t3")
                        nc.vector.tensor_add(t3[:], s01[:], s23[:])
                    elif variant == "acts":
                        # muls: 2 DVE ts + 2 ACT; adds: DVE, POOL, DVE
                        ms = []
                        for j in range(K):
                            m = tpool.tile([128, TC], F32, tag=f"m{j}")
                            if j < 2:
                                nc.vector.tensor_scalar_mul(m[:], xslice(j), kap(j))
                            else:
                                nc.scalar.mul(m[:], xslice(j), kap(j))
                            ms.append(m)
                        s01 = tpool.tile([128, TC], F32, tag="s01")
                        nc.vector.tensor_add(s01[:], ms[0][:], ms[1][:])
                        s23 = tpool.tile([128, TC], F32, tag="s23")
                        nc.gpsimd.tensor_add(s23[:], ms[2][:], ms[3][:])
                        t3 = tpool.tile([128, TC], F32, tag="t3")
                        nc.vector.tensor_add(t3[:], s01[:], s23[:])
                    elif variant == "v4":
                        # TC=4096, in-place accumulation, DVE/ACT/POOL split:
                        #   DVE: m0=x0*k0, m1=x1*k1, m0+=m1, m0+=m2
                        #   ACT: m2=x2*k2, m3=x3*k3, y=silu(m0)
                        #   POOL: m2+=m3
                        m0 = tpool.tile([128, TC], F32, tag="m0")
                        nc.vector.tensor_scalar_mul(m0[:], xslice(0), kap(0))
                        m1 = tpool.tile([128, TC], F32, tag="m1")
                        nc.vector.tensor_scalar_mul(m1[:], xslice(1), kap(1))
                        m2 = tpool.tile([128, TC], F32, tag="m2")
                        nc.scalar.mul(m2[:], xslice(2), kap(2))
                        m3 = tpool.tile([128, TC], F32, tag="m3")
                        nc.scalar.mul(m3[:], xslice(3), kap(3))
                        nc.vector.tensor_add(m0[:], m0[:], m1[:])
                        nc.gpsimd.tensor_add(m2[:], m2[:], m3[:])
                        nc.vector.tensor_add(m0[:], m0[:], m2[:])
                        t3 = m0
                    else:
                        raise ValueError(f"unknown variant {variant}")
                    if variant == "v4":
                        y_sb = t3  # silu in place, DMA out of the same tile
                    else:
                        y_sb = ypool.tile([128, TC], F32)
                    if use_silu:
                        nc.scalar.activation(
                            y_sb[:], t3[:], mybir.ActivationFunctionType.Silu
                        )
                    else:
                        # CoreSim does not implement Silu; emulate as z*sigmoid(z)
                        sg = tpool.tile([128, TC], F32, tag="sg")
                        nc.scalar.activation(
                            sg[:], t3[:], mybir.ActivationFunctionType.Sigmoid
                        )
                        if variant == "v4":
                            y_sb = ypool.tile([128, TC], F32)
                        nc.vector.tensor_mul(y_sb[:], t3[:], sg[:])
                    nc.sync.dma_start(
                        ys_ap[rows, it * TC : (it + 1) * TC], y_sb[:]
                    )

    nc.compile()
    _PROG_CACHE[key] = nc
    return nc


def _shard_inputs(x: np.ndarray, kern: np.ndarray):
    """Full [B,T,C] inputs -> 8 per-core {xs, ks} maps (channel sharding)."""
    # xs_all rows ordered r = c*B + b; columns: [PAD zeros | x[b, :, c]]
    xs_all = np.empty((C * B, PAD + T), dtype=np.float32)
    xs_all[:, :PAD] = 0.0
    # view of the payload as [C, B, T]; strided scatter from x^T
    xs_all[:, PAD:].reshape(C, B, T)[:] = x.transpose(2, 0, 1)
    ks_all = np.repeat(kern.T, B, axis=0)  # [C*B, K], row r = c*B + b
    ks_all = np.ascontiguousarray(ks_all, dtype=np.float32)
    in_maps = [
        {
            "xs": xs_all[i * ROWS : (i + 1) * ROWS],
            "ks": ks_all[i * ROWS : (i + 1) * ROWS],
        }
        for i in range(NCORES)
    ]
    return in_maps


def kernel(x: np.ndarray, kernel: np.ndarray):
    from concourse import bass_utils

    x = np.ascontiguousarray(x, dtype=np.float32)
    kern = np.ascontiguousarray(kernel, dtype=np.float32)

    variant = os.environ.get("CONV_VARIANT", "split")
    nc = _build_program(use_silu=True, variant=variant)
    in_maps = _shard_inputs(x, kern)

    trace = os.environ.get("CONV_TRACE", "0") == "1"
    res = bass_utils.run_bass_kernel_spmd(
        nc, in_maps, list(range(NCORES)), trace=trace
    )
    globals()["LAST_RESULTS"] = res  # for the test harness (exec_time_ns etc.)

    ys_all = np.concatenate([out["ys"] for out in res.results], axis=0)
    y = np.empty((B, T, C), dtype=np.float32)
    y.transpose(2, 0, 1)[:] = ys_all.reshape(C, B, T)

    next_cache = np.ascontiguousarray(x[:, T - PAD :, :])
    return y, next_cache


# revision 22
# speedup vs baseline: 1.3697x; 1.3697x over previous
"""Trainium2 Bass kernel: causal depthwise short conv1d + SiLU.

Problem: x [B=4, T=4096, C=2048] f32, kernel [K=4, C=2048] f32.
  y[b, t, c] = silu(sum_j kernel[j, c] * x[b, t - j, c])   (zero left-pad)
  next_cache = x[:, T-K+1:, :]

Strategy:
  - Each (b, c) pair is an independent length-T sequence -> B*C = 8192 rows.
  - Shard channels across the 8 cores: core i handles channels
    [i*256, (i+1)*256) -> 1024 rows of [PAD + T] (zero pre-padded).
  - On-chip layout: partition = row, free dim = time. The 4 taps become
    per-partition tensor_scalar / scalar_tensor_tensor fused mul-adds on
    DVE/GPSIMD; SiLU runs on the scalar engine.
"""

import os
import sys

import numpy as np

_TRN_REPO = "/opt/trn_rl_repo"
if _TRN_REPO not in sys.path:
    sys.path.insert(0, _TRN_REPO)

B, T, C, K = 4, 4096, 2048, 4
PAD = K - 1
NCORES = 8
CPC = C // NCORES       # 256 channels per core
ROWS = CPC * B          # 1024 rows per core

_PROG_CACHE: dict = {}


def _build_program(use_silu: bool = True, variant: str = "split", loop: int = 1):
    """Build the per-core Bass program (identical on all 8 cores).

    loop > 1 wraps the whole body in a hardware For_i that redoes the
    (idempotent) work `loop` times — benchmarking only.
    """
    key = (use_silu, variant, loop)
    if key in _PROG_CACHE:
        return _PROG_CACHE[key]

    import concourse.tile as tile
    from concourse import bacc, mybir

    nc = bacc.Bacc("TRN2")
    xs = nc.dram_tensor("xs", [ROWS, PAD + T], mybir.dt.float32, kind="ExternalInput")
    ks = nc.dram_tensor("ks", [ROWS, K], mybir.dt.float32, kind="ExternalInput")
    peg = int(variant[2:]) if variant[:2] in ("pe", "pr") else 0
    use_f32r = variant.startswith("pr")
    if peg:
        # per row-group diagonal weight matrices diag(k_j) for the PE path
        ws = nc.dram_tensor(
            "ws", [ROWS // 128, K, 128, 128], mybir.dt.float32, kind="ExternalInput"
        )
    ys = nc.dram_tensor("ys", [ROWS, T], mybir.dt.float32, kind="ExternalOutput")

    MULT = mybir.AluOpType.mult
    ADD = mybir.AluOpType.add
    F32 = mybir.dt.float32

    xs_ap, ks_ap, ys_ap = xs.ap(), ks.ap(), ys.ap()

    from contextlib import ExitStack

    TC = 4096 if variant == "v4" else 2048
    # row-groups handled by the tensor engine (diag-matmul conv)
    pe_sets = {2: (0, 4), 3: (0, 3, 6), 4: (0, 2, 4, 6), 5: (0, 2, 3, 5, 7),
               6: (0, 1, 2, 4, 5, 6), 8: tuple(range(8))}
    pe_groups = set(pe_sets.get(peg, ())) if peg else set()

    with tile.TileContext(nc) as tc:
        with ExitStack() as stack:
            # sim mode (use_silu=False) only checks numerics; bufs=1 keeps
            # the extra sigmoid tile inside the SBUF budget
            bx = (3 if TC == 2048 else 2) if use_silu else 1
            bt = 2 if use_silu else 1
            by = (3 if TC == 2048 else 1) if use_silu else 1
            kpool = stack.enter_context(tc.tile_pool(name="kp", bufs=2))
            xpool = stack.enter_context(tc.tile_pool(name="xp", bufs=bx))
            tpool = stack.enter_context(tc.tile_pool(name="tp", bufs=bt))
            ypool = stack.enter_context(tc.tile_pool(name="yp", bufs=by))
            if peg:
                wpool = stack.enter_context(tc.tile_pool(name="wp", bufs=2))
                ppool = stack.enter_context(
                    tc.tile_pool(name="pp", bufs=4, space="PSUM")
                )
                ws_ap = ws.ap()
            if loop > 1:
                stack.enter_context(tc.For_i(0, loop, 1))
            for r in range(ROWS // 128):
                rows = slice(r * 128, (r + 1) * 128)
                k_sb = kpool.tile([128, K], F32)
                nc.sync.dma_start(k_sb[:], ks_ap[rows, :])
                if r in pe_groups:
                    w_sb = wpool.tile([128, K * 128], F32)
                    for j in range(K):
                        nc.sync.dma_start(
                            w_sb[:, j * 128 : (j + 1) * 128],
                            ws_ap[r, j].rearrange("p m -> p m"),
                        )
                for it in range(T // TC):
                    x_sb = xpool.tile([128, TC + PAD], F32)
                    nc.sync.dma_start(
                        x_sb[:], xs_ap[rows, it * TC : it * TC + TC + PAD]
                    )
                    if r in pe_groups:
                        # conv via 4 accumulating diag-matmuls per PSUM bank
                        y_sb = ypool.tile([128, TC], F32)
                        for b in range(TC // 512):
                            ps = ppool.tile([128, 512], F32)
                            for j in range(K):
                                off = PAD - j + b * 512
                                lhsT = w_sb[:, j * 128 : (j + 1) * 128]
                                rhs = x_sb[:, off : off + 512]
                                if use_f32r:
                                    lhsT = lhsT.bitcast(mybir.dt.float32r)
                                    rhs = rhs.bitcast(mybir.dt.float32r)
                                nc.tensor.matmul(
                                    ps[:],
                                    lhsT=lhsT,
                                    rhs=rhs,
                                    start=(j == 0),
                                    stop=(j == K - 1),
                                )
                            sl = slice(b * 512, (b + 1) * 512)
                            if use_silu:
                                nc.scalar.activation(
                                    y_sb[:, sl], ps[:],
                                    mybir.ActivationFunctionType.Silu,
                                )
                            else:
                                sg = tpool.tile([128, 512], F32, tag="psg")
                                nc.scalar.activation(
                                    sg[:], ps[:],
                                    mybir.ActivationFunctionType.Sigmoid,
                                )
                                nc.vector.tensor_mul(y_sb[:, sl], ps[:], sg[:])
                        nc.sync.dma_start(
                            ys_ap[rows, it * TC : (it + 1) * TC], y_sb[:]
                        )
                        continue
                    if variant == "dmaonly":
                        nc.sync.dma_start(
                            ys_ap[rows, it * TC : (it + 1) * TC],
                            x_sb[:, PAD : PAD + TC],
                        )
                        continue
                    # z = sum_j k[:, j] * x[t - j]; x_sb col (PAD + t) = x[t]
                    def xslice(j):
                        return x_sb[:, PAD - j : PAD - j + TC]

                    def kap(j):
                        return k_sb[:, j : j + 1]

                    if variant in ("dve", "split"):
                        # chain of fused (x*k)+acc on DVE (tap2 optionally Pool)
                        t0 = tpool.tile([128, TC], F32, tag="t0")
                        nc.vector.tensor_scalar_mul(t0[:], xslice(0), kap(0))
                        t1 = tpool.tile([128, TC], F32, tag="t1")
                        nc.vector.scalar_tensor_tensor(
                            t1[:], xslice(1), kap(1), t0[:], MULT, ADD
                        )
                        t2 = tpool.tile([128, TC], F32, tag="t2")
                        eng2 = nc.gpsimd if variant == "split" else nc.vector
                        eng2.scalar_tensor_tensor(
                            t2[:], xslice(2), kap(2), t1[:], MULT, ADD
                        )
                        t3 = tpool.tile([128, TC], F32, tag="t3")
                        nc.vector.scalar_tensor_tensor(
                            t3[:], xslice(3), kap(3), t2[:], MULT, ADD
                        )
                    elif variant == "tt" or peg:
                        # 4 tensor_scalar muls + add tree, all DVE
                        ms = []
                        for j in range(K):
                            m = tpool.tile([128, TC], F32, tag=f"m{j}")
                            nc.vector.tensor_scalar_mul(m[:], xslice(j), kap(j))
                            ms.append(m)
                        s01 = tpool.tile([128, TC], F32, tag="s01")
                        nc.vector.tensor_add(s01[:], ms[0][:], ms[1][:])
                        s23 = tpool.tile([128, TC], F32, tag="s23")
                        nc.vector.tensor_add(s23[:], ms[2][:], ms[3][:])
                        t3 = tpool.tile([128, TC], F32, tag="t3")
                        nc.vector.tensor_add(t3[:], s01[:], s23[:])
                    elif variant == "acts":
                        # muls: 2 DVE ts + 2 ACT; adds: DVE, POOL, DVE
                        ms = []
                        for j in range(K):
                            m = tpool.tile([128, TC], F32, tag=f"m{j}")
                            if j < 2:
                                nc.vector.tensor_scalar_mul(m[:], xslice(j), kap(j))
                            else:
                                nc.scalar.mul(m[:], xslice(j), kap(j))
                            ms.append(m)
                        s01 = tpool.tile([128, TC], F32, tag="s01")
                        nc.vector.tensor_add(s01[:], ms[0][:], ms[1][:])
                        s23 = tpool.tile([128, TC], F32, tag="s23")
                        nc.gpsimd.tensor_add(s23[:], ms[2][:], ms[3][:])
                        t3 = tpool.tile([128, TC], F32, tag="t3")
                        nc.vector.tensor_add(t3[:], s01[:], s23[:])
                    elif variant == "v4":
                        # TC=4096, in-place accumulation, DVE/ACT/POOL split:
                        #   DVE: m0=x0*k0, m1=x1*k1, m0+=m1, m0+=m2
                        #   ACT: m2=x2*k2, m3=x3*k3, y=silu(m0)
                        #   POOL: m2+=m3
                        m0 = tpool.tile([128, TC], F32, tag="m0")
                        nc.vector.tensor_scalar_mul(m0[:], xslice(0), kap(0))
                        m1 = tpool.tile([128, TC], F32, tag="m1")
                        nc.vector.tensor_scalar_mul(m1[:], xslice(1), kap(1))
                        m2 = tpool.tile([128, TC], F32, tag="m2")
                        nc.scalar.mul(m2[:], xslice(2), kap(2))
                        m3 = tpool.tile([128, TC], F32, tag="m3")
                        nc.scalar.mul(m3[:], xslice(3), kap(3))
                        nc.vector.tensor_add(m0[:], m0[:], m1[:])
                        nc.gpsimd.tensor_add(m2[:], m2[:], m3[:])
                        nc.vector.tensor_add(m0[:], m0[:], m2[:])
                        t3 = m0
                    else:
                        raise ValueError(f"unknown variant {variant}")
                    if variant == "v4":
                        y_sb = t3  # silu in place, DMA out of the same tile
                    else:
                        y_sb = ypool.tile([128, TC], F32)
                    if use_silu:
                        nc.scalar.activation(
                            y_sb[:], t3[:], mybir.ActivationFunctionType.Silu
                        )
                    else:
                        # CoreSim does not implement Silu; emulate as z*sigmoid(z)
                        sg = tpool.tile([128, TC], F32, tag="sg")
                        nc.scalar.activation(
                            sg[:], t3[:], mybir.ActivationFunctionType.Sigmoid
                        )
                        if variant == "v4":
                            y_sb = ypool.tile([128, TC], F32)
                        nc.vector.tensor_mul(y_sb[:], t3[:], sg[:])
                    nc.sync.dma_start(
                        ys_ap[rows, it * TC : (it + 1) * TC], y_sb[:]
                    )

    nc.compile()
    _PROG_CACHE[key] = nc
    return nc


def _shard_inputs(x: np.ndarray, kern: np.ndarray):
    """Full [B,T,C] inputs -> 8 per-core {xs, ks, ws} maps (channel sharding)."""
    # xs_all rows ordered r = c*B + b; columns: [PAD zeros | x[b, :, c]]
    xs_all = np.empty((C * B, PAD + T), dtype=np.float32)
    xs_all[:, :PAD] = 0.0
    # view of the payload as [C, B, T]; strided scatter from x^T
    xs_all[:, PAD:].reshape(C, B, T)[:] = x.transpose(2, 0, 1)
    ks_all = np.repeat(kern.T, B, axis=0)  # [C*B, K], row r = c*B + b
    ks_all = np.ascontiguousarray(ks_all, dtype=np.float32)
    # diag weight matrices for the PE path: ws[i][g, j] = diag(ks[g*128:(g+1)*128, j])
    ng = ROWS // 128
    idx = np.arange(128)
    in_maps = []
    for i in range(NCORES):
        ksi = ks_all[i * ROWS : (i + 1) * ROWS]
        wsi = np.zeros((ng, K, 128, 128), dtype=np.float32)
        for g in range(ng):
            for j in range(K):
                wsi[g, j, idx, idx] = ksi[g * 128 : (g + 1) * 128, j]
        in_maps.append(
            {
                "xs": xs_all[i * ROWS : (i + 1) * ROWS],
                "ks": ksi,
                "ws": wsi,
            }
        )
    return in_maps


def kernel(x: np.ndarray, kernel: np.ndarray):
    from concourse import bass_utils

    x = np.ascontiguousarray(x, dtype=np.float32)
    kern = np.ascontiguousarray(kernel, dtype=np.float32)

    variant = os.environ.get("CONV_VARIANT", "split")
    nc = _build_program(use_silu=True, variant=variant)
    in_maps = _shard_inputs(x, kern)

    trace = os.environ.get("CONV_TRACE", "0") == "1"
    res = bass_utils.run_bass_kernel_spmd(
        nc, in_maps, list(range(NCORES)), trace=trace
    )
    globals()["LAST_RESULTS"] = res  # for the test harness (exec_time_ns etc.)

    ys_all = np.concatenate([out["ys"] for out in res.results], axis=0)
    y = np.empty((B, T, C), dtype=np.float32)
    y.transpose(2, 0, 1)[:] = ys_all.reshape(C, B, T)

    next_cache = np.ascontiguousarray(x[:, T - PAD :, :])
    return y, next_cache
